# revision 9
# baseline (speedup 1.0000x reference)
"""Bidirectional DTW (symmetric2, L1) batch kernel for Trainium2, 8 cores.

64 pairs of length-1024 fp32 sequences; per pair the full 1024x1024 DTW
DP; output = mean over pairs of D[n-1,m-1] / (n + m).

Strategy: halve the serial DP depth by running the forward DP (rows
0..511) and a backward DP (rows 1023..512, as a transformed forward DP
on reversed sequences) CONCURRENTLY, then merging on the host at the
row-511/512 cut:

    total = min_j( F[511,j] + H[512,1023-j] - |x[511]-y[j]| )

where H[a,b] = G[a,b] + dt[a,b] is the backward-exclusive cost DP in
reversed coordinates; H obeys the same 3-op row recurrence as F except
the diagonal add uses the PREVIOUS row's dt:

    F:  p_j = min(Dprev[j-1] + d[j],   Dprev[j]);  D = scan(min,add)(p, d)
    H:  p_b = min(Hprev[b-1] + e[b-1], Hprev[b]);  H = scan(min,add)(p, dt)
        (e = previous row's dt)

Per core: 8 samples x {fwd,bwd} = 16 virtual wavefronts, each split
into C=8 column chunks of W=128: partitions p = 8c+b (fwd),
64+8c+b (bwd). Chunks run in a software wavefront with a 2-row skew;
the chunk->chunk boundary column moves through the TensorEngine every
step (shift matmul into PSUM; the next-next step's scan reads its
initial carry STRAIGHT from PSUM - no copy op).

Per step: 2 ACT ops produce the two d-rows (the bwd half needs both
dt[a] and dt[a-1], with a one-column shift on the u-operand - handled
entirely by separate Y/X staging tiles so every compute op stays one
uniform [128,*] instruction); DVE does add, min, 129-wide
tensor_tensor_scan. 527 steps total vs 1264 for a unidirectional
64-col-chunk wavefront.
"""

import sys

sys.path.insert(0, "/opt/trn_rl_repo")

import numpy as np

import concourse.bass as bass
import concourse.bacc as bacc
import concourse.mybir as mybir
from concourse import tile
from concourse.bass_utils import run_bass_kernel_spmd

AF = mybir.ActivationFunctionType
ALU = mybir.AluOpType
FP32 = mybir.dt.float32
FP16 = mybir.dt.float16

NCORES = 8
B = 8              # samples per core
N = 1024           # sequence length (rows == cols)
C = 8              # column chunks per direction
W = N // C         # 128 columns per chunk
SKEW = 2           # rows of lag between adjacent chunks
RF = 512           # forward rows: 0..511
RB = 513           # backward rows: a = 0..512
T = RB + SKEW * (C - 1)   # 527 steps
NSLOT = 16         # row-state arena slots (keeps final rows alive)
BIG = 1.0e30

_CACHE = {}


def _build():
    nc = bacc.Bacc("TRN2", target_bir_lowering=False, debug=False)
    xa_in = nc.declare_dram_parameter("xa", [128, T], FP32, isOutput=False)
    xb_in = nc.declare_dram_parameter("xb", [128, T], FP32, isOutput=False)
    ya_in = nc.declare_dram_parameter("ya", [128, W], FP16, isOutput=False)
    yb_in = nc.declare_dram_parameter("yb", [128, W], FP16, isOutput=False)
    s8_in = nc.declare_dram_parameter("s8", [128, 128], FP32, isOutput=False)
    rows_out = nc.declare_dram_parameter("rows", [128, W], FP32, isOutput=True)

    with tile.TileContext(nc) as tc:
        with (
            tc.tile_pool(name="persist", bufs=1) as pp,
            tc.tile_pool(name="psum", bufs=4, space=bass.MemorySpace.PSUM) as psp,
        ):
            S8 = pp.tile([128, 128], FP32, tag="s8t")
            W2 = pp.tile([128, 128], FP32, tag="w2t")
            ONES = pp.tile([128, 1], FP32, tag="ones")
            ZC = pp.tile([128, 1], FP32, tag="zc")
            XA = pp.tile([128, T], FP32, tag="xa")
            XB = pp.tile([128, T], FP32, tag="xb")
            YA = pp.tile([128, W], FP16, tag="ya")
            YB = pp.tile([128, W], FP16, tag="yb")
            BB = pp.tile([128, NSLOT, W + 1], FP32, tag="bb")

            nc.sync.dma_start(S8[:], s8_in[:])
            nc.sync.dma_start(XA[:], xa_in[:])
            nc.sync.dma_start(XB[:], xb_in[:])
            nc.sync.dma_start(YA[:], ya_in[:])
            nc.sync.dma_start(YB[:], yb_in[:])

            nc.vector.memset(BB[:], BIG)
            # zero-carry at t=0 for chunk-0 partitions of both halves
            nc.vector.memset(ZC[:], BIG)
            nc.vector.memset(ZC[0:8, :], 0.0)
            nc.vector.memset(ZC[64:72, :], 0.0)
            # BIG-patch matmul weights: acc[p] += BIG for chunk-0 partitions
            nc.vector.memset(W2[:], 0.0)
            nc.vector.memset(W2[0:1, 0:8], BIG)
            nc.vector.memset(W2[0:1, 64:72], BIG)
            nc.vector.memset(ONES[:], 1.0)

            # d rings: DB feeds the scan (col 0 = 0 forever); DA feeds the
            # u-add (col 0 never read)
            DAs = [pp.tile([128, W + 1], FP16, name=f"da{i}", tag=f"da{i}") for i in range(4)]
            DBs = [pp.tile([128, W + 1], FP16, name=f"db{i}", tag=f"db{i}") for i in range(4)]
            for t_ in DBs:
                nc.vector.memset(t_[:, 0:1], 0.0)
            # p tiles: col 0 = BIG forever (scan lead-through writes carry)
            pts = [pp.tile([128, W + 1], FP32, name=f"pt{i}", tag=f"pt{i}") for i in range(3)]
            uts = [pp.tile([128, W], FP32, name=f"ut{i}", tag=f"ut{i}") for i in range(3)]
            for t_ in pts:
                nc.vector.memset(t_[:, 0:1], BIG)

            for t in range(T):
                da = DAs[t % 4]
                db = DBs[t % 4]
                nc.scalar.activation(
                    da[:, 1 : W + 1], YA[:], AF.Abs, bias=XA[:, t : t + 1], scale=1.0
                )
                nc.scalar.activation(
                    db[:, 1 : W + 1], YB[:], AF.Abs, bias=XB[:, t : t + 1], scale=1.0
                )
                b_prev = BB[:, (t - 1) % NSLOT, :]
                b_cur = BB[:, t % NSLOT, :]
                u = uts[t % 3]
                p = pts[t % 3]
                nc.vector.tensor_tensor(
                    u[:, 0:W], b_prev[:, 0:W], da[:, 1 : W + 1], op=ALU.add
                )
                nc.vector.tensor_tensor(
                    p[:, 1 : W + 1], u[:, 0:W], b_prev[:, 1 : W + 1], op=ALU.min
                )
                if t == 0:
                    init_ap = ZC[:, 0:1]
                else:
                    acc = psp.tile([128, 1], FP32, tag="acc", name="acc")
                    nc.tensor.matmul(
                        acc[:, 0:1],
                        S8[:],
                        BB[:, (t - 2) % NSLOT, W : W + 1],
                        start=True,
                        stop=False,
                    )
                    nc.tensor.matmul(
                        acc[:, 0:1],
                        W2[:],
                        ONES[:, 0:1],
                        start=False,
                        stop=True,
                        skip_group_check=True,
                    )
                    init_ap = acc[:, 0:1]
                nc.vector.tensor_tensor_scan(
                    b_cur[:, 0 : W + 1],
                    p[:, 0 : W + 1],
                    db[:, 0 : W + 1],
                    init_ap,
                    op0=ALU.min,
                    op1=ALU.add,
                )
                if t == 0:
                    # the zero carry must not persist as next row's
                    # Dprev[-1]: the grid's left edge is +inf, and row 1
                    # would otherwise see an illegal diagonal "D[0,-1]=0"
                    nc.vector.memset(b_cur[:, 0:1], BIG)

            for c in range(C):
                slot_f = (RF - 1 + SKEW * c) % NSLOT
                slot_b = (RB - 1 + SKEW * c) % NSLOT
                nc.sync.dma_start(
                    rows_out[8 * c : 8 * c + 8, :],
                    BB[8 * c : 8 * c + 8, slot_f, 1 : W + 1],
                )
                nc.sync.dma_start(
                    rows_out[64 + 8 * c : 64 + 8 * c + 8, :],
                    BB[64 + 8 * c : 64 + 8 * c + 8, slot_b, 1 : W + 1],
                )

    nc.compile()
    return nc


def _shift_matrix():
    # out[p] = in[p-8] within each 64-partition half; rows that would
    # cross the fwd/bwd boundary stay zero (chunk-0 carries come from the
    # BIG patch instead).
    s8 = np.zeros((128, 128), np.float32)
    for r in range(120):
        p = r + 8
        if (r // 64) == (p // 64):
            s8[r, p] = 1.0
    return s8


def _stage_inputs(xs: np.ndarray, ys: np.ndarray):
    """Build XA/XB/YA/YB staging for one core (xs, ys: [8, 1024] fp32)."""
    xf = xs[:, :RF]                       # fwd rows 0..511
    xr = xs[:, ::-1][:, :RB]              # reversed, rows a = 0..512
    yr = ys[:, ::-1]

    XB = np.full((128, T), BIG, np.float32)
    XA = np.full((128, T), BIG, np.float32)
    YB = np.empty((128, W), np.float32)
    YA = np.empty((128, W), np.float32)
    for c in range(C):
        for b in range(B):
            pf = 8 * c + b
            pb = 64 + 8 * c + b
            o = SKEW * c
            # fwd: row i = t - o
            XB[pf, o : o + RF] = xf[b]
            XA[pf, o : o + RF] = xf[b]
            # bwd: row a = t - o  (XB), a-1 = t - o - 1 (XA)
            XB[pb, o : o + RB] = xr[b]
            XA[pb, o + 1 : min(o + 1 + RB, T)] = xr[b][: T - o - 1]
            YB[pf] = ys[b, W * c : W * c + W]
            YA[pf] = YB[pf]
            YB[pb] = yr[b, W * c : W * c + W]
            # YA bwd: shifted by one column (eps indexing); col 0 of
            # chunk 0 is never meaningful (carry is BIG there)
            if c == 0:
                YA[pb, 0] = 0.0
                YA[pb, 1:] = yr[b, 0 : W - 1]
            else:
                YA[pb] = yr[b, W * c - 1 : W * c + W - 1]
    # activation computes |scale*y + bias| with bias = -x
    return -XA, -XB, YA.astype(np.float16), YB.astype(np.float16)


def kernel(x: np.ndarray, x_target: np.ndarray) -> np.ndarray:
    x = np.ascontiguousarray(np.asarray(x, np.float32))
    y = np.ascontiguousarray(np.asarray(x_target, np.float32))
    if "nc" not in _CACHE:
        _CACHE["nc"] = _build()
    nc = _CACHE["nc"]
    s8 = _shift_matrix()
    in_maps = []
    for k in range(NCORES):
        xs = x[8 * k : 8 * k + 8]
        ys = y[8 * k : 8 * k + 8]
        XA, XB, YA, YB = _stage_inputs(xs, ys)
        in_maps.append({"xa": XA, "xb": XB, "ya": YA, "yb": YB, "s8": s8})
    bres = run_bass_kernel_spmd(nc, in_maps, list(range(NCORES)))
    _CACHE["last_results"] = bres

    dists = np.empty(64, np.float64)
    for k in range(NCORES):
        rows = np.asarray(bres.results[k]["rows"], np.float64)  # [128, W]
        for b in range(B):
            F_row = np.empty(N)
            H_row = np.empty(N)
            for c in range(C):
                F_row[W * c : W * c + W] = rows[8 * c + b]
                H_row[W * c : W * c + W] = rows[64 + 8 * c + b]
            xs = np.float64(x[8 * k + b, RF - 1])        # x[511]
            ys_ = np.asarray(x_target[8 * k + b], np.float64)
            B_r = H_row[::-1] - np.abs(xs - ys_)         # H[512,1023-j] - d[511,j]
            dists[8 * k + b] = (F_row + B_r).min() / (2.0 * N)
    return np.float32(dists.mean())
